# revision 54
# baseline (speedup 1.0000x reference)
"""Trainium2 Bass kernel for nn_Actor_56916906607124 (compute_encoder_mask).

Computation (per batch instance b, row i):
  mask[b,i,j] = 1 iff  (j is among the 16 nearest time-window-compatible,
                        non-diagonal neighbors of i)  OR depot[b,i]  OR
                        depot[b,j]  OR i == j.

Sharding: pure data parallelism — batch B=8 across 8 NeuronCores, one
instance per core.  No collectives.

Per-core algorithm (N=2048, 16 row-tiles of 128 rows):
  selection key  x = twc - d  (eligible j: x in (0,1]; blocked j: x <= 0);
  diagonal forced out of selection (x -= 2.5 on the diag block).
  16th-largest x per row (DVE): 4 chunk-wise `max` (top-8 of each 512-col
  chunk) -> 32 candidates, global `max`, `match_replace` removes those 8,
  `max` again -> v8b[:,7] = 16th largest.  A chunk holding >8 of the true
  top-16 under-estimates the threshold, which the count check catches.
  knn = (x >= t16) in bf16 with a per-row count (accum_out) shipped out.
  The ORs run on the otherwise-idle engines: the TensorEngine accumulates
  knn + depot_col + diag-identity into PSUM via identity matmuls (sums in
  {0..3}, exact), and the Scalar engine's Sign LUT with a per-partition
  bias of 2*depot_row emits the final 0/1 mask directly as uint8 (the host
  widens to f32 while unsharding -- the store shrinks 4x).
  Mask stores are issued on the sync HWDGE queue after every load so loads
  stream at pure rate; the first tiles load in column pieces to start the
  DVE early.
  Host repairs rows whose count != 16 or whose raw threshold is <= 0
  (ignoring depot rows, which are all-ones by construction) by exact numpy
  reference recomputation -- float ties at the 8/9 or 16/17 selection
  boundary, chunk mis-coverage, or <16 eligible neighbors.  O(N) per
  flagged row; ~450 of 16384 rows on the seed-0 data, dominated by the
  4-chunk coverage heuristic, all repaired exactly.
"""

from contextlib import ExitStack

import numpy as np

import concourse.bass as bass
import concourse.mybir as mybir
from concourse import bacc, tile

B, N, P = 8, 2048, 128
NT = N // P  # 16 row-tiles per core
K = 16
f32 = mybir.dt.float32
i32 = mybir.dt.int32
bf16 = mybir.dt.bfloat16
Alu = mybir.AluOpType
Act = mybir.ActivationFunctionType

_program_cache = {}


def build_program():
    if "nc" in _program_cache:
        return _program_cache["nc"]
    nc = bacc.Bacc()
    d_h = nc.declare_dram_parameter("d", [N, N], f32, isOutput=False)
    twc_h = nc.declare_dram_parameter("twc", [N, N], i32, isOutput=False)
    dflat_h = nc.declare_dram_parameter("dflat", [1, N], bf16, isOutput=False)
    drow2_h = nc.declare_dram_parameter("drow2", [P, NT], f32, isOutput=False)
    ident_h = nc.declare_dram_parameter("ident", [P, P], bf16, isOutput=False)
    mask_h = nc.declare_dram_parameter("mask", [N, N], mybir.dt.uint8,
                                       isOutput=True)
    nge_h = nc.declare_dram_parameter("nge", [P, NT], f32, isOutput=True)
    t16r_h = nc.declare_dram_parameter("t16r", [P, NT], f32, isOutput=True)

    with ExitStack() as ctx:
        tc = ctx.enter_context(tile.TileContext(nc))
        const = ctx.enter_context(tc.tile_pool(name="const", bufs=1))
        inp = ctx.enter_context(tc.tile_pool(name="inp", bufs=5))
        work = ctx.enter_context(tc.tile_pool(name="work", bufs=4))
        outp = ctx.enter_context(tc.tile_pool(name="outp", bufs=16))
        small = ctx.enter_context(tc.tile_pool(name="small", bufs=6))
        psum = ctx.enter_context(
            tc.tile_pool(name="psum", bufs=4, space="PSUM"))

        # build dc01 (depot broadcast across partitions) on-chip: K=1 matmul
        # ones[1,P].T @ depot[1,N-chunk] replicates the depot row to all
        # partitions; the idle Scalar engine narrows PSUM f32 -> SBUF bf16
        dflat_s = const.tile([1, N], bf16)
        nc.sync.dma_start(dflat_s[:], dflat_h[:, :])
        ones_s = const.tile([1, P], bf16)
        nc.gpsimd.memset(ones_s[:], 1.0)
        dc01_s = const.tile([P, N], bf16)
        for c in range(4):
            cols = slice(c * 512, (c + 1) * 512)
            pt = psum.tile([P, 512], f32, tag="pb")
            nc.tensor.matmul(pt[:], ones_s[:], dflat_s[:, cols])
            nc.scalar.activation(dc01_s[:, cols], pt[:], Act.Copy)
        drow2_s = const.tile([P, NT], f32)
        nc.sync.dma_start(drow2_s[:], drow2_h[:, :])
        ident_s = const.tile([P, P], bf16)
        nc.sync.dma_start(ident_s[:], ident_h[:, :])
        nge_s = const.tile([P, NT], f32)
        t16r_s = const.tile([P, NT], f32)

        NCH = 4          # selection chunks per row
        CW = N // NCH    # 512 columns per chunk
        pending_stores = []
        for r in range(NT):
            rows = slice(r * P, (r + 1) * P)
            d_t = inp.tile([P, N], f32, tag="d")
            twc_t = inp.tile([P, N], i32, tag="twc")
            x = work.tile([P, N], f32, tag="x")
            if r <= 1:
                # first two tiles: load + build x in column pieces so the
                # DVE starts right after the first half-megabyte and is not
                # starved while the load stream ramps
                np_ = 4 if r == 0 else 2
                for h in range(np_):
                    cs = slice(h * (N // np_), (h + 1) * (N // np_))
                    nc.sync.dma_start(d_t[:, cs], d_h[rows, cs])
                    nc.sync.dma_start(twc_t[:, cs], twc_h[rows, cs])
                    nc.vector.tensor_tensor(
                        x[:, cs], twc_t[:, cs], d_t[:, cs], Alu.subtract)
            else:
                nc.sync.dma_start(d_t[:], d_h[rows, :])
                nc.sync.dma_start(twc_t[:], twc_h[rows, :])
                # x = twc - d: eligible j have x in (0,1], blocked j have
                # x <= 0, so the top-16 of x = the 16 nearest eligible
                nc.vector.tensor_tensor(x[:], twc_t[:], d_t[:], Alu.subtract)
            # exclude diagonal from selection: x_diag -= 2.5
            xblk = x[:, rows]
            nc.vector.scalar_tensor_tensor(
                xblk, ident_s[:], -2.5, xblk, Alu.mult, Alu.add
            )
            # per-chunk top-8 -> 64 candidates.  The true top-16 is contained
            # in the candidates unless one 256-chunk holds >8 of it; that rare
            # case makes the computed threshold strictly smaller, so the row
            # count comes out > 16 and the host repairs the row exactly.
            cand = small.tile([P, NCH * 8], f32, tag="cand")
            for c in range(NCH):
                nc.vector.max(cand[:, c * 8 : (c + 1) * 8],
                              x[:, c * CW : (c + 1) * CW])
            # global top-8 (always exact: a chunk top-8 covers its share)
            v8a = small.tile([P, 8], f32, tag="v8a")
            nc.vector.max(v8a[:], cand[:])
            # remove exactly those 8 from the candidates, then next-8
            cand2 = small.tile([P, NCH * 8], f32, tag="cand2")
            nc.vector.match_replace(cand2[:], v8a[:], cand[:], -1e30)
            v8b = small.tile([P, 8], f32, tag="v8b")
            nc.vector.max(v8b[:], cand2[:])
            # knn = (x >= 16th largest), nge[:, r] = per-row count
            # (diag still excluded: x_diag <= -0.5 < t16, so the count is a
            #  pure top-16 count -- 16 unless a float tie at a boundary)
            # t16' = 16th largest - 1e30*depot_row: depot rows compare all-true
            # (whole row is 1 in the reference), and the host ignores their
            # count when flagging tie rows.
            # raw 16th-largest shipped out: t16r <= 0 means the row had fewer
            # than 16 eligible neighbors (never on this data; host repairs)
            nc.vector.tensor_copy(t16r_s[:, r : r + 1], v8b[:, 7:8])
            knn = work.tile([P, N], bf16, tag="knn")
            if r == NT - 1:
                # last tile: compare in halves so the PE/ACT/store drain
                # starts while the second half is still comparing
                na = small.tile([P, 1], f32, tag="na")
                nb = small.tile([P, 1], f32, tag="nb")
                half = N // 2
                nc.vector.tensor_scalar(
                    knn[:, :half], x[:, :half], v8b[:, 7:8], None,
                    Alu.is_ge, Alu.add, accum_out=na[:])
                nc.vector.tensor_scalar(
                    knn[:, half:], x[:, half:], v8b[:, 7:8], None,
                    Alu.is_ge, Alu.add, accum_out=nb[:])
                nc.vector.tensor_tensor(
                    nge_s[:, r : r + 1], na[:], nb[:], Alu.add)
            else:
                nc.vector.tensor_scalar(
                    knn[:], x[:], v8b[:, 7:8], None, Alu.is_ge, Alu.add,
                    accum_out=nge_s[:, r : r + 1],
                )
            # out = knn OR depot_col on the idle engines: identity-matmul
            # accumulate knn + dc01 into PSUM (sums in {0,1,2}, exact), then
            # the Scalar engine's Sign LUT emits the 0/1 mask as uint8
            # (host widens to f32) -- the store shrinks 4x and the DVE is
            # freed of the OR pass entirely.
            # force the diagonal on (after the count accum; tiny bf16 2x op)
            kblk = knn[:, rows]
            nc.vector.tensor_tensor(kblk, kblk, ident_s[:], Alu.logical_or)
            out_t = outp.tile([P, N], mybir.dt.uint8, tag="out")
            for c in range(4):
                cols = slice(c * 512, (c + 1) * 512)
                pt = psum.tile([P, 512], f32, tag="acc")
                nc.tensor.matmul(pt[:], ident_s[:], knn[:, cols],
                                 start=True, stop=False)
                nc.tensor.matmul(pt[:], ident_s[:], dc01_s[:, cols],
                                 start=False, stop=True)
                nc.scalar.activation(out_t[:, cols], pt[:], Act.Sign,
                                     bias=drow2_s[:, r : r + 1])
            pending_stores.append((rows, out_t))

        # all mask stores issued on the sync queue AFTER every load: the
        # HWDGE FIFO then gives loads strict priority, so the load stream
        # runs at pure rate and compute never starves; stores fill the
        # DMA idle time at the end of the stream.
        for rows, out_t in pending_stores[:-1]:
            nc.sync.dma_start(mask_h[rows, :], out_t[:])
        # the final tile's store rides the otherwise-idle scalar queue so it
        # needn't wait behind the 15 queued sync-stores at the drain
        rows, out_t = pending_stores[-1]
        nc.scalar.dma_start(mask_h[rows, :], out_t[:])
        nc.scalar.dma_start(nge_h[:, :], nge_s[:])
        nc.scalar.dma_start(t16r_h[:, :], t16r_s[:])

    nc.compile()
    _program_cache["nc"] = nc
    return nc


def _repair_row(d_row, twc_row, depot_b, max_dist_b, i):
    """Exact float32 re-computation of reference row i (handles ties)."""
    n = d_row.shape[0]
    m = (twc_row == 0).astype(np.float32)
    m[i] = np.float32(1.0)
    big = (m * np.float32(max_dist_b)) * np.float32(10.0)
    dist = d_row * (np.float32(1.0) - m) + big
    idx = np.argsort(dist, kind="stable")[:K]
    knn = np.zeros(n, np.float32)
    knn[idx] = 1.0
    knn *= (twc_row == 1)
    dep = (depot_b + depot_b[i]) > 0
    out = ((knn > 0) | dep | (np.arange(n) == i)).astype(np.float32)
    return out


def make_in_maps(distance_matrix, time_window_compatibility, depot):
    bf = mybir.dt.np(bf16)
    ident = np.eye(P, dtype=bf)
    in_maps = []
    for b in range(B):
        dep_f = depot[b].astype(np.float32)
        in_maps.append({
            "d": distance_matrix[b],
            "twc": time_window_compatibility[b],
            "dflat": np.ascontiguousarray(dep_f.astype(bf).reshape(1, N)),
            "drow2": np.ascontiguousarray(
                (dep_f * np.float32(2.0)).reshape(NT, P).T),
            "ident": ident,
        })
    return in_maps


def _get_executor():
    """Build the 8-core shard_map executable once (mirrors
    bass2jax.run_bass_via_pjrt, but cached so repeat calls skip retracing)."""
    if "exec" in _program_cache:
        return _program_cache["exec"]
    import jax
    from jax.sharding import Mesh, NamedSharding, PartitionSpec
    from jax.experimental.shard_map import shard_map
    from concourse import bass2jax
    from concourse.bass2jax import _bass_exec_p, install_neuronx_cc_hook

    nc = build_program()
    install_neuronx_cc_hook()
    partition_name = (nc.partition_id_tensor.name
                      if nc.partition_id_tensor else None)
    in_names, out_names, out_avals = [], [], []
    for alloc in nc.m.functions[0].allocations:
        if not isinstance(alloc, mybir.MemoryLocationSet):
            continue
        name = alloc.memorylocations[0].name
        if alloc.kind == "ExternalInput":
            if name != partition_name:
                in_names.append(name)
        elif alloc.kind == "ExternalOutput":
            out_names.append(name)
            out_avals.append(jax.core.ShapedArray(
                tuple(alloc.tensor_shape), mybir.dt.np(alloc.dtype)))
    all_in_names = list(in_names) + list(out_names)
    if partition_name is not None:
        all_in_names.append(partition_name)

    def _body(*args):
        operands = list(args)
        if partition_name is not None:
            operands.append(bass2jax.partition_id_tensor())
        return tuple(_bass_exec_p.bind(
            *operands,
            out_avals=tuple(out_avals),
            in_names=tuple(all_in_names),
            out_names=tuple(out_names),
            lowering_input_output_aliases=(),
            sim_require_finite=True,
            sim_require_nnan=True,
            nc=nc,
        ))

    devices = jax.devices()[:B]
    mesh = Mesh(np.asarray(devices), ("core",))
    spec = PartitionSpec("core")
    n_io = len(in_names) + len(out_names)
    sharded = jax.jit(
        shard_map(_body, mesh=mesh, in_specs=(spec,) * n_io,
                  out_specs=(spec,) * len(out_names), check_rep=False),
        donate_argnums=tuple(range(len(in_names), n_io)), keep_unused=True,
    )
    sharding = NamedSharding(mesh, spec)
    ex = (sharded, in_names, out_names, out_avals, sharding)
    _program_cache["exec"] = ex
    return ex


def _run_device(args_dev):
    import jax

    sharded, in_names, out_names, out_avals, sharding = _get_executor()
    # the kernel fully overwrites all outputs; donate last call's buffers
    prev = _program_cache.get("outs")
    if prev is None:
        prev = tuple(jax.device_put(
            np.zeros((B * av.shape[0], *av.shape[1:]), av.dtype), sharding)
            for av in out_avals)
    outs_dev = sharded(*args_dev, *prev)
    _program_cache["outs"] = outs_dev
    return {n: np.array(a).reshape(B, *out_avals[i].shape)
            for i, (n, a) in enumerate(zip(out_names, outs_dev))}


def kernel(distance_matrix, max_dist, time_window_compatibility, depot,
           num_neighbors_encoder):
    import jax

    distance_matrix = np.asarray(distance_matrix, dtype=np.float32)
    time_window_compatibility = np.asarray(time_window_compatibility,
                                           dtype=np.int32)
    depot = np.asarray(depot, dtype=np.int32)
    max_dist = np.asarray(max_dist, dtype=np.float32).reshape(B)
    assert int(np.asarray(num_neighbors_encoder)) == K
    assert distance_matrix.shape == (B, N, N)

    sharded, in_names, out_names, out_avals, sharding = _get_executor()
    in_maps = make_in_maps(distance_matrix, time_window_compatibility, depot)
    concat_in = [np.concatenate([in_maps[c][n] for c in range(B)], axis=0)
                 for n in in_names]
    args_dev = [jax.device_put(a, sharding) for a in concat_in]

    rng = np.random.default_rng(0)
    for attempt in range(3):
        by_name = _run_device(args_dev)
        out = by_name["mask"].astype(np.float32)  # widen device's 0/1 uint8
        nge = by_name["nge"]      # [B, P, NT]
        t16r = by_name["t16r"]

        # exact repair of rows with a float tie at a selection boundary, or
        # with fewer than 16 eligible neighbors (t16r <= 0).  Depot rows are
        # all-ones by construction and never need repair.
        flag = ((nge != np.float32(K)) | (t16r <= 0)) & (
            depot.reshape(B, NT, P).transpose(0, 2, 1) == 0)
        for b, p, r in zip(*np.nonzero(flag)):
            i = int(r) * P + int(p)
            out[b, i] = _repair_row(
                distance_matrix[b, i], time_window_compatibility[b, i],
                depot[b], max_dist[b], i,
            )

        # audit: recompute a random sample of rows exactly on host; any
        # mismatch indicates a transient device glitch -> rerun the call
        ok = True
        for _ in range(192):
            b = int(rng.integers(B))
            i = int(rng.integers(N))
            exp = _repair_row(
                distance_matrix[b, i], time_window_compatibility[b, i],
                depot[b], max_dist[b], i,
            )
            if not np.array_equal(out[b, i], exp):
                ok = False
                break
        if ok:
            return out
    return out
